# revision 19
# baseline (speedup 1.0000x reference)
"""2-layer GCN (nn_Discriminator2) on 8 Trainium2 NeuronCores via Bass/Tile.

v3 decomposition (dest-sharded graph parallel):
  conv1: h = x @ W1 per local node shard, table h~ = dis*h in fp8e4m3
         (512B rows, declared bf16[256] for DMA/collective compat),
         AllGather; edges (self-loops excluded) sorted by dest and diced
         into 128-edge tiles per 128-dest block; each tile is one fp8 PE
         matmul psum += S.T @ msg with dma_gather-fetched rows. Self-loop
         term dis^2*h is added locally in the epilogue.
  conv2: z = h1 @ W2 as a DVE row-dot per block, z~ stored as bf16
         (z, 0) pairs; TWO half AllGathers (first issued mid-conv1 so it
         hides under the gathers), DMA-broadcast into a [128, N] SBUF
         table, per-dest segment sums via ap_gather (8 Q7 cores in
         parallel, per-core streams) + tensor_reduce over degree tiers.

SPMD constraint: one instruction stream for all 8 cores -> tile counts and
tier shapes are maxed across cores; cores pad with zero-S rows / dummy idx.
"""

import math
import numpy as np
import ml_dtypes

BF16 = ml_dtypes.bfloat16
FP8 = ml_dtypes.float8_e4m3
EPS = 1e-3
P = 128          # partitions / dest-block size / tile lane count
NGRP = 8         # GPSIMD cores (16-partition groups)
NTIER = 8        # conv2 degree tiers


# ----------------------------------------------------------------------------
# Host-side graph preprocessing (structure only: indices, binary selectors)
# ----------------------------------------------------------------------------

def preprocess(edge_index: np.ndarray, n: int, ncores: int):
    """Balanced packing: permute dest nodes across (core, block) bins so
    per-bin edge counts are near-equal; the SPMD cross-core max then adds ~no
    padding. slot_of[v] = global slot of node v; device arrays follow it."""
    src = edge_index[0].astype(np.int64)
    dst = edge_index[1].astype(np.int64)
    deg = np.bincount(dst, minlength=n).astype(np.int64) + 1   # incl self-loop
    dis = (1.0 / np.sqrt(deg.astype(np.float64))).astype(np.float32)

    nblk = max(1, math.ceil(n / ncores / P))
    nlocp = nblk * P
    nbins = ncores * nblk

    # deal nodes (ranked by degree desc) snake-wise across bins, cap P each
    order = np.argsort(-deg, kind="stable")
    bincnt = np.zeros(nbins, np.int64)
    slot_of = np.zeros(n, np.int64)
    bin_members = [[] for _ in range(nbins)]
    bi = 0
    direction = 1
    for v in order:
        tries = 0
        while bincnt[bi] >= P:
            bi += direction
            if bi == nbins:
                bi = nbins - 1; direction = -1
            elif bi < 0:
                bi = 0; direction = 1
            tries += 1
            assert tries <= 2 * nbins
        bin_members[bi].append(v)
        bincnt[bi] += 1
        bi += direction
        if bi == nbins:
            bi = nbins - 1; direction = -1
        elif bi < 0:
            bi = 0; direction = 1
    for b in range(nbins):
        core, blk = divmod(b, nblk)
        base = core * nlocp + blk * P
        for s_, v in enumerate(bin_members[b]):
            slot_of[v] = base + s_

    # ---------------- conv1 tiles: edges only (no self-loops) --------------
    dpos = slot_of[dst]
    spos = slot_of[src]
    core = dpos // nlocp
    rem = dpos - core * nlocp
    blk = rem // P
    wloc = rem % P

    cnt = np.zeros((ncores, nblk), np.int64)
    np.add.at(cnt, (core, blk), 1)
    T = np.maximum(1, -(-cnt // P)).max(axis=0)          # [nblk]
    slot_base_b = np.zeros(nblk, np.int64)
    slot_base_b[1:] = np.cumsum(T)[:-1]
    tot = int(T.sum())

    key = core * nblk + blk
    order_e = np.argsort(key, kind="stable")
    c_o, b_o, wl_o, sp_o = (core[order_e], blk[order_e],
                            wloc[order_e], spos[order_e])
    key_o = key[order_e]
    first = np.r_[True, key_o[1:] != key_o[:-1]]
    idx_arr = np.arange(len(key_o))
    grp_start = np.maximum.accumulate(np.where(first, idx_arr, 0))
    seq = idx_arr - grp_start
    tile_k = seq // P
    jj = seq % P
    slot = slot_base_b[b_o] + tile_k
    assert (tile_k < T[b_o]).all()

    idx16 = np.zeros((ncores, tot * P), np.int16)
    idx16[c_o, slot * P + jj] = sp_o.astype(np.int16)
    S8 = np.zeros((ncores, P, tot * P), np.uint8)
    S8[c_o, jj, slot * P + wl_o] = np.float32(1.0).astype(FP8).view(np.uint8)
    # pack fp8 pairs into bf16-shaped carrier for I/O
    Sp = np.ascontiguousarray(S8).view(np.uint16).view(BF16)  # [nc, P, tot*64]

    # wrapped int16 index layout: slot-major i -> [i % 16, i // 16],
    # replicated into all 8 GPSIMD core partition groups
    idxw = np.tile(idx16.reshape(ncores, tot * 8, 16).transpose(0, 2, 1),
                   (1, 8, 1)).copy()

    # ---------------- conv2 segments (ap_gather tiers) --------------------
    nseg = nlocp // NGRP                      # dests per group
    cut = 16 * P                              # first 16 blocks in AllGather A
    rest = nlocp - cut
    ztab_pad = ncores * nlocp                 # index of a guaranteed-zero pair

    def zpos(s):
        """table position of global slot s with split AllGather layout"""
        c, l = s // nlocp, s % nlocp
        return np.where(l < cut, c * cut + l,
                        ncores * cut + c * rest + (l - cut))

    # in-edge src slots per dest slot
    eorder = np.argsort(dpos, kind="stable")
    dpos_s = dpos[eorder]
    spos_s = spos[eorder]
    seg_start = np.searchsorted(dpos_s, np.arange(ncores * nlocp))
    seg_end = np.searchsorted(dpos_s, np.arange(ncores * nlocp) + 1)
    seglen_all = (seg_end - seg_start + 1)    # + self

    # deal dests to groups by degree rank (snake) for balance
    c2dest = np.zeros((ncores, NGRP, nseg), np.int64)
    for c in range(ncores):
        slots = np.arange(c * nlocp, (c + 1) * nlocp)
        lens = seglen_all[slots]
        o = np.argsort(-lens, kind="stable")
        so = slots[o]
        for r in range(nseg):
            row = so[r * NGRP:(r + 1) * NGRP]
            if r % 2:
                row = row[::-1]
            c2dest[c, :, r] = row

    c2len = seglen_all[c2dest]                # [ncores, NGRP, nseg]
    # rank-wise max over (core, group) -> tier boundaries via DP
    M = c2len.max(axis=(0, 1))                # [nseg] non-increasing
    INF = 1 << 60
    dp = np.full((NTIER + 1, nseg + 1), INF, np.int64)
    par = np.zeros((NTIER + 1, nseg + 1), np.int64)
    dp[0, 0] = 0
    for t in range(1, NTIER + 1):
        for e in range(1, nseg + 1):
            costs = dp[t - 1, :e] + (e - np.arange(e)) * M[:e]
            s = int(np.argmin(costs))
            if costs[s] < dp[t, e]:
                dp[t, e] = costs[s]
                par[t, e] = s
    bounds = [nseg]
    t, e = NTIER, nseg
    while t > 0 and e > 0:
        s = int(par[t, e])
        bounds.append(s)
        t, e = t - 1, s
    bounds = sorted(set(bounds + [0]))
    tiers = [(bounds[i], bounds[i + 1], int(M[bounds[i]]))
             for i in range(len(bounds) - 1)]

    tot2 = sum((e - s) * K for s, e, K in tiers)
    tot2p = tot2 + ((-tot2) % 16)

    idx2 = np.full((ncores, P, tot2p // 16), ztab_pad, np.int16)
    out_order = np.zeros(ncores * nlocp, np.int64)  # final[sl] = out[order[sl]]
    for c in range(ncores):
        for g in range(NGRP):
            lst = np.full(tot2p, ztab_pad, np.int64)
            off = 0
            for s_, e_, K in tiers:
                for r in range(s_, e_):
                    d = c2dest[c, g, r]
                    ln = seglen_all[d]
                    lst[off:off + ln - 1] = zpos(spos_s[seg_start[d]:seg_end[d]])
                    lst[off + ln - 1] = zpos(np.int64(d))
                    off += K
            idx2[c, 16 * g:16 * g + 16, :] = \
                lst.reshape(tot2p // 16, 16).T.astype(np.int16)
            out_order[c2dest[c, g]] = c * nlocp + g * nseg + np.arange(nseg)

    return dict(
        dis=dis, nloc=nlocp, nblk=nblk, nlocp=nlocp, tot=tot,
        T=T, idxw=idxw, S=Sp, slot_of=slot_of,
        tiers=tiers, tot2p=tot2p, idx2=idx2, nseg=nseg,
        c2dest=c2dest, out_order=out_order, ztab_pad=ztab_pad,
    )


# ----------------------------------------------------------------------------
# Bass program
# ----------------------------------------------------------------------------

def build_program(n, f, ncores, nblk, nlocp, tot, T, nloc,
                  tiers, tot2p, nseg):
    import concourse.bacc as bacc
    import concourse.mybir as mybir
    import concourse.tile as tile

    f2 = f // 2                    # 134: fp8 row bytes / 2 (bf16 carrier cols)
    fpad8 = 512                    # fp8 row padded: 512 B, %256 == 0
    cpad = fpad8 // 2              # 256 bf16-carrier cols
    kch = -(-f // P)               # contraction chunks for x @ W1
    ag_rows = ncores * nlocp
    cut = 16 * P
    rest = nlocp - cut
    ztab_n = ag_rows + P           # z table entries (+ zero pad region)
    dt = mybir.dt
    Alu = mybir.AluOpType
    Act = mybir.ActivationFunctionType

    nc = bacc.Bacc("TRN2", target_bir_lowering=False, debug=False,
                   num_devices=ncores)

    xT_in = nc.dram_tensor("xT", [kch * P, nlocp], dt.bfloat16, kind="ExternalInput")
    W1_in = nc.dram_tensor("W1p", [kch * P, f], dt.bfloat16, kind="ExternalInput")
    vecs_in = nc.dram_tensor("vecs", [6, f], dt.float32, kind="ExternalInput")
    scal_in = nc.dram_tensor("scal", [1, 8], dt.float32, kind="ExternalInput")
    dis_in = nc.dram_tensor("disb", [P, nblk], dt.float32, kind="ExternalInput")
    dis2_in = nc.dram_tensor("dis2w", [P, nseg], dt.float32, kind="ExternalInput")
    idx_in = nc.dram_tensor("idxw", [P, tot * 8], dt.int16, kind="ExternalInput")
    idx2_in = nc.dram_tensor("idx2", [P, tot2p // 16], dt.int16, kind="ExternalInput")
    S_in = nc.dram_tensor("S", [P, tot * 64], dt.bfloat16, kind="ExternalInput")
    out_ext = nc.dram_tensor("out", [nloc, 1], dt.float32, kind="ExternalOutput")

    shared = "Shared" if ncores > 4 else "Local"
    h_loc = nc.dram_tensor("h_loc", [nlocp, cpad], dt.bfloat16)
    h_ag = nc.dram_tensor("h_ag", [ag_rows, cpad], dt.bfloat16, addr_space=shared)
    z_loc = nc.dram_tensor("z_loc", [nlocp, 2], dt.bfloat16)
    z_ag_a = nc.dram_tensor("z_ag_a", [1, ncores * cut * 2], dt.bfloat16,
                            addr_space=shared)
    z_ag_b = nc.dram_tensor("z_ag_b", [1, ncores * rest * 2], dt.bfloat16,
                            addr_space=shared)

    rg = [list(range(ncores))]

    with tile.TileContext(nc) as tc:
        with (
            tc.tile_pool(name="const", bufs=1) as cp,
            tc.tile_pool(name="work", bufs=3) as wp,
            tc.tile_pool(name="msgp", bufs=3) as mp,
            tc.tile_pool(name="gath", bufs=1) as gp,
            tc.tile_pool(name="psum", bufs=4, space="PSUM") as pp,
            tc.tile_pool(name="psum1", bufs=2, space="PSUM") as pp1,
            tc.tile_pool(name="psumr", bufs=1, space="PSUM") as ppr,
        ):
            # ---------------- constants ----------------
            W1_sb = cp.tile([P, kch, f], dt.bfloat16)
            nc.sync.dma_start(out=W1_sb[:], in_=W1_in.ap().rearrange("(k p) n -> p k n", p=P))
            scal = cp.tile([1, 8], dt.float32)
            nc.sync.dma_start(out=scal[:], in_=scal_in[:])
            dis_sb = cp.tile([P, nblk], dt.float32)
            nc.sync.dma_start(out=dis_sb[:], in_=dis_in[:])
            dis2_sb = cp.tile([P, nseg], dt.float32)
            nc.sync.dma_start(out=dis2_sb[:], in_=dis2_in[:])
            idx_sb = cp.tile([P, tot * 8], dt.int16)
            nc.sync.dma_start(out=idx_sb[:], in_=idx_in[:])
            idx2_sb = cp.tile([P, tot2p // 16], dt.int16)
            nc.sync.dma_start(out=idx2_sb[:], in_=idx2_in[:])
            S_sb = cp.tile([P, tot * 64], dt.bfloat16)
            nc.sync.dma_start(out=S_sb[:], in_=S_in[:])
            S8 = S_sb[:].bitcast(dt.float8e4)            # [P, tot*128]

            ones = cp.tile([1, P], dt.float32)
            nc.vector.memset(ones[:], 1.0)

            # zero the padded tail columns of the gather table (one DMA)
            zpad = cp.tile([P, 1, cpad - f2], dt.bfloat16)
            nc.vector.memset(zpad[:], 0.0)
            nc.sync.dma_start(
                out=h_loc.ap()[:, f2:].rearrange("(b p) c -> p b c", p=P),
                in_=zpad[:].broadcast_to([P, nblk, cpad - f2]),
            )

            vrows = cp.tile([1, 6, f], dt.float32)
            nc.sync.dma_start(
                out=vrows[:],
                in_=vecs_in.ap().rearrange("(o r) n -> o r n", o=1))

            # k1 = g1 / sqrt(rv1 + eps); t1 = beta1 - rm1 * k1      [1, f]
            k1 = cp.tile([1, f], dt.float32)
            t1 = cp.tile([1, f], dt.float32)
            tmp = cp.tile([1, f], dt.float32)
            nc.vector.tensor_scalar_add(tmp[:], vrows[:, 4, :], EPS)
            nc.scalar.sqrt(tmp[:], tmp[:])
            nc.vector.reciprocal(tmp[:], tmp[:])
            nc.vector.tensor_tensor(out=k1[:], in0=tmp[:], in1=vrows[:, 1, :], op=Alu.mult)
            nc.vector.tensor_tensor(out=tmp[:], in0=vrows[:, 3, :], in1=k1[:], op=Alu.mult)
            nc.vector.tensor_tensor(out=t1[:], in0=vrows[:, 2, :], in1=tmp[:], op=Alu.subtract)

            # k2 = g2 / sqrt(rv2 + eps); t2 = beta2 - rm2 * k2; pack [1,4]
            sc_row = cp.tile([1, 4], dt.float32)
            nc.vector.memset(sc_row[:], 0.0)
            stmp = cp.tile([1, 1], dt.float32)
            nc.vector.tensor_copy(out=sc_row[:, 0:1], in_=scal[:, 0:1])           # b2
            nc.vector.tensor_scalar_add(stmp[:], scal[:, 4:5], EPS)
            nc.scalar.sqrt(stmp[:], stmp[:])
            nc.vector.reciprocal(stmp[:], stmp[:])
            nc.vector.tensor_tensor(out=sc_row[:, 1:2], in0=stmp[:], in1=scal[:, 1:2], op=Alu.mult)  # k2
            nc.vector.tensor_tensor(out=stmp[:], in0=scal[:, 3:4], in1=sc_row[:, 1:2], op=Alu.mult)
            nc.vector.tensor_tensor(out=sc_row[:, 2:3], in0=scal[:, 2:3], in1=stmp[:], op=Alu.subtract)  # t2

            def replicate(row_ap, width):
                ps = ppr.tile([P, width], dt.float32, tag="rep")
                nc.tensor.matmul(out=ps[:], lhsT=ones[:], rhs=row_ap, start=True, stop=True)
                sb = cp.tile([P, width], dt.float32, tag=f"rep{replicate.i}")
                replicate.i += 1
                nc.vector.tensor_copy(out=sb[:], in_=ps[:])
                return sb
            replicate.i = 0

            B1rep = replicate(vrows[:, 0, :], f)
            K1rep = replicate(k1[:], f)
            T1rep = replicate(t1[:], f)
            W2rep = replicate(vrows[:, 5, :], f)
            SCrep = replicate(sc_row[:], 4)      # cols: b2, k2, t2

            # local raw h kept for the self-loop term (fp8)
            h_keep = cp.tile([P, nblk, f], dt.float8e4)

            # pre-generate descriptors for the first NPREP gathers on the
            # GPSIMD engine while phase1 + the AllGather occupy PE/CC; the
            # data dep on h_ag defers to trigger_dma (Tile handles it).
            # conv2 z table (allocated early: its first half is broadcast
            # mid-conv1, right after the first z AllGather)
            ztab = gp.tile([P, ztab_n * 2], dt.bfloat16)
            nc.vector.memset(ztab[:, ag_rows * 2:], 0.0)

            NPREP = 0

            # ---------------- phase 1: h~ = dis * (x @ W1) ----------------
            for nb in range(nblk):
                xb = wp.tile([P, kch, P], dt.bfloat16, tag="xb")
                nc.sync.dma_start(
                    out=xb[:],
                    in_=xT_in.ap().rearrange("(k p) n -> p k n", p=P)[
                        :, :, nb * P:(nb + 1) * P],
                )
                ps = pp1.tile([P, f], dt.float32, tag="ps1")
                for kc in range(kch):
                    nc.tensor.matmul(
                        out=ps[:],
                        lhsT=xb[:, kc, :],
                        rhs=W1_sb[:, kc, :],
                        start=(kc == 0), stop=(kc == kch - 1),
                    )
                nc.vector.tensor_copy(out=h_keep[:, nb, :], in_=ps[:])
                hb = wp.tile([P, f], dt.float8e4, tag="hb")
                nc.vector.tensor_scalar_mul(hb[:], ps[:], dis_sb[:, nb:nb + 1])
                nc.sync.dma_start(out=h_loc[nb * P:(nb + 1) * P, :f2],
                                  in_=hb[:].bitcast(dt.bfloat16))

            nc.gpsimd.collective_compute(
                "AllGather", Alu.bypass, replica_groups=rg,
                ins=[h_loc[:]], outs=[h_ag[:]],
            )

            # ---------------- conv1 aggregation + epilogue -> z~ ----------
            for b in range(nblk):
                base = int(T[:b].sum()) if b else 0
                ntile = int(T[b])
                msg = mp.tile([P, ntile, cpad], dt.bfloat16, tag="msg")
                nc.gpsimd.dma_gather(
                    out_ap=msg[:],
                    in_ap=h_ag[:],
                    idxs_ap=idx_sb[:, base * 8:(base + ntile) * 8],
                    num_idxs=ntile * P,
                    num_idxs_reg=ntile * P,
                    elem_size=cpad,
                    single_packet=False,
                    queue_num=0,
                )
                msg8 = msg[:].bitcast(dt.float8e4)   # [P, ntile, fpad8]
                ps = pp.tile([P, f], dt.float32, tag="ps")
                for k in range(ntile):
                    nc.tensor.matmul(
                        out=ps[:],
                        lhsT=S8[:, (base + k) * P:(base + k + 1) * P],
                        rhs=msg8[:, k, :f],
                        start=(k == 0), stop=(k == ntile - 1),
                    )

                # epilogue: u = dis*(agg + dis*h) + b1; relu; *k1+t1; relu
                sl = wp.tile([P, f], dt.float32, tag="sl")
                nc.vector.tensor_scalar_mul(sl[:], h_keep[:, b, :], dis_sb[:, b:b + 1])
                u = wp.tile([P, f], dt.float32, tag="u")
                nc.vector.tensor_tensor(out=u[:], in0=ps[:], in1=sl[:], op=Alu.add)
                nc.vector.tensor_scalar_mul(u[:], u[:], dis_sb[:, b:b + 1])
                nc.vector.tensor_tensor(out=u[:], in0=u[:], in1=B1rep[:], op=Alu.add)
                nc.scalar.activation(u[:], u[:], Act.Relu)
                nc.vector.tensor_tensor(out=u[:], in0=u[:], in1=K1rep[:], op=Alu.mult)
                nc.vector.tensor_tensor(out=u[:], in0=u[:], in1=T1rep[:], op=Alu.add)
                nc.scalar.activation(u[:], u[:], Act.Relu)
                # z~ = dis * (h1 . W2) as (z, 0) bf16 pairs
                zt = wp.tile([P, f], dt.float32, tag="zt")
                nc.vector.tensor_tensor(out=zt[:], in0=u[:], in1=W2rep[:], op=Alu.mult)
                zr = wp.tile([P, 1], dt.float32, tag="zr")
                nc.vector.tensor_reduce(out=zr[:], in_=zt[:], axis=mybir.AxisListType.X, op=Alu.add)
                zz = wp.tile([P, 2], dt.bfloat16, tag="zz")
                nc.vector.memset(zz[:], 0.0)
                nc.vector.tensor_scalar_mul(zz[:, 0:1], zr[:], dis_sb[:, b:b + 1])
                nc.sync.dma_start(out=z_loc[b * P:(b + 1) * P, :], in_=zz[:])

                if b == 18:
                    nc.gpsimd.collective_compute(
                        "AllGather", Alu.bypass, replica_groups=rg,
                        ins=[z_loc[0:cut, :]], outs=[z_ag_a[:]],
                    )
                    nc.sync.dma_start(
                        out=ztab[:, :ncores * cut * 2],
                        in_=z_ag_a.ap().broadcast_to([P, ncores * cut * 2]),
                    )

            nc.gpsimd.collective_compute(
                "AllGather", Alu.bypass, replica_groups=rg,
                ins=[z_loc[cut:, :]], outs=[z_ag_b[:]],
            )

            # ---------------- conv2: second-part z broadcast + ap_gather --
            nc.sync.dma_start(
                out=ztab[:, ncores * cut * 2:ag_rows * 2],
                in_=z_ag_b.ap().broadcast_to([P, ncores * rest * 2]),
            )

            gath = gp.tile([P, tot2p * 2], dt.bfloat16)
            nc.gpsimd.ap_gather(
                out_ap=gath[:],
                in_ap=ztab[:],
                idxs_ap=idx2_sb[:],
                channels=P,
                num_elems=ztab_n,
                d=2,
                num_idxs=tot2p,
            )

            red = gp.tile([P, nseg], dt.float32)
            off = 0
            for s_, e_, K in tiers:
                nt = e_ - s_
                nc.vector.tensor_reduce(
                    out=red[:, s_:e_],
                    in_=gath[:, off * 2:(off + nt * K) * 2]
                        .rearrange("p (nn kk) -> p nn kk", kk=2 * K),
                    axis=mybir.AxisListType.X, op=Alu.add,
                )
                off += nt * K

            # out2 = sigmoid(relu(k2*(dis*red + b2) + t2))
            nc.vector.tensor_tensor(out=red[:], in0=red[:], in1=dis2_sb[:], op=Alu.mult)
            nc.vector.tensor_scalar_add(red[:], red[:], SCrep[:, 0:1])
            nc.vector.tensor_scalar_mul(red[:], red[:], SCrep[:, 1:2])
            nc.vector.tensor_scalar_add(red[:], red[:], SCrep[:, 2:3])
            o = gp.tile([P, nseg], dt.float32)
            nc.scalar.activation(o[:], red[:], Act.Relu)
            nc.scalar.activation(o[:], o[:], Act.Sigmoid)
            for g in range(NGRP):
                nc.sync.dma_start(
                    out=out_ext[g * nseg:(g + 1) * nseg, :].rearrange("n one -> one n"),
                    in_=o[16 * g:16 * g + 1, :],
                )

    nc.compile()
    return nc


# ----------------------------------------------------------------------------
# Full pipeline
# ----------------------------------------------------------------------------

def make_inputs(x, W1, b1, g1, beta1, rm1, rv1, W2, b2, g2, beta2, rm2, rv2,
                pre, ncores):
    n, f = x.shape
    nlocp, nblk, nseg = pre["nlocp"], pre["nblk"], pre["nseg"]
    slot_of = pre["slot_of"]
    kch = -(-f // P)
    fpk = kch * P

    W1p = np.zeros((fpk, f), BF16)
    W1p[:f, :] = W1.astype(BF16)
    vecs = np.stack([b1, g1, beta1, rm1, rv1, W2[:, 0]]).astype(np.float32)
    scal = np.zeros((1, 8), np.float32)
    scal[0, :5] = [b2[0], g2[0], beta2[0], rm2[0], rv2[0]]

    dis = pre["dis"]
    core_of = slot_of // nlocp
    local = slot_of - core_of * nlocp
    dis_by_slot = np.zeros(ncores * nlocp, np.float32)
    dis_by_slot[slot_of] = dis

    in_maps = []
    for c in range(ncores):
        sel = core_of == c
        loc = local[sel]
        xT = np.zeros((fpk, nlocp), BF16)
        xT[:f, loc] = x[sel].T.astype(BF16)
        db = np.zeros(nlocp, np.float32)
        db[loc] = dis[sel]
        disb = db.reshape(nblk, P).T.copy()
        d2 = dis_by_slot[pre["c2dest"][c]]          # [NGRP, nseg]
        dis2w = np.repeat(d2, 16, axis=0).astype(np.float32)  # [128, nseg]
        in_maps.append({
            "xT": xT, "W1p": W1p, "vecs": vecs, "scal": scal,
            "disb": disb, "dis2w": dis2w,
            "idxw": pre["idxw"][c], "idx2": pre["idx2"][c], "S": pre["S"][c],
        })
    return in_maps


def _install_ntff_hook():
    """bass_utils wants antenv.axon_hooks for trace=True under axon; this
    container's antenv lacks it. Inject a shim backed by the boot helper."""
    import sys, types
    if "antenv.axon_hooks" in sys.modules:
        return
    try:
        from trn_agent_boot.trn_boot import _ntff_profile_via_ctypes
        hook = _ntff_profile_via_ctypes("/opt/axon/libaxon_pjrt.so")
    except Exception:
        hook = None
    mod = types.ModuleType("antenv.axon_hooks")
    mod.get_axon_ntff_profile_hook = lambda: hook
    mod.set_axon_ntff_profile_hook = lambda h: None
    sys.modules["antenv.axon_hooks"] = mod


def run(inputs, ncores=8, trace=False, tmpdir=None):
    from concourse.bass_utils import run_bass_kernel_spmd
    if trace:
        _install_ntff_hook()

    x = np.asarray(inputs["x"])
    n, f = x.shape
    pre = preprocess(np.asarray(inputs["edge_index"]), n, ncores)
    nc = build_program(n, f, ncores, pre["nblk"], pre["nlocp"], pre["tot"],
                       pre["T"], pre["nloc"],
                       pre["tiers"], pre["tot2p"], pre["nseg"])
    in_maps = make_inputs(
        x, *(np.asarray(inputs[k]) for k in
             ["W1", "b1", "g1", "beta1", "rm1", "rv1",
              "W2", "b2", "g2", "beta2", "rm2", "rv2"]),
        pre, ncores)
    res = run_bass_kernel_spmd(nc, in_maps, list(range(ncores)), trace=trace,
                               tmpdir=tmpdir)
    allout = np.concatenate([res.results[c]["out"] for c in range(ncores)], axis=0)
    out = allout[pre["out_order"][pre["slot_of"]]]
    return out, res, pre, nc


# ----------------------------------------------------------------------------
# Harness entry point: full inputs in, full output out.
# ----------------------------------------------------------------------------

def kernel(**inputs) -> np.ndarray:
    out, _res, _pre, _nc = run(inputs, ncores=8, trace=False)
    return out.astype(np.float32)
